# revision 9
# baseline (speedup 1.0000x reference)
"""MoE expert-collection grouped GEMM for Trainium2, expert-parallel over 8
NeuronCores, fp8 DoubleRow matmuls.

Problem (hardcoded shapes):
  sorted_features  [65536, 1024] f32   tokens sorted by expert, 4096/expert
  expert_ids_sorted[65536] i32         unused: split is static equal-count
  routing_matrix   [1024, 2048, 16] f32
  routing_bias     [2048, 16] f32
  out = silu(x_e @ W_e + b_e) per expert  -> [65536, 2048] f32

Sharding: expert-parallel, 2 experts (= 8192 contiguous sorted tokens) per
core. Host-side dispatch hands each core its token block transposed
(feature-major) and scaled by S_X in fp8 e4m3, its 2 experts' weights scaled
by S_W in fp8 e4m3 (DRAM pre-laid in SBUF tile order: 4KB per-partition
lines), and bias pre-scaled by S_X*S_W broadcast to 128 partitions in fp16.
The matmul term has std ~0.17 vs bias std 1.0, so fp8 noise lands well
under the 2e-2 gate.

Device pipeline per core: DoubleRow fp8 matmuls (two 128-row k-blocks per
instruction, 2 MACs/cell/cycle) accumulating fp32 in PSUM, k-pair-outer /
out-block-inner so one stationary x tile serves 4 matmuls. A tsub's 4 PSUM
banks accumulate while the other 4 drain through one batched DVE add
(+prescaled fp16 bias) and one batched ACT Silu (scale=1/(S_X*S_W))
writing fp16; host upcasts to fp32.

Schedule shaping:
- 6 zero-matmul warmups flip the PE HAM clock-gate to 2.4GHz while the
  critical preload streams, so real matmuls start warm.
- weight/x loads are need-ordered across the two HWDGE rings; expert-0
  bias is deferred behind a dependency on w-kp1 so its descriptors don't
  steal head-window queue slots from the critical weight stream.
- y stores ride the scalar ring (a store on the sync ring blocks the
  in-order sequencer and delays x prefetch issue) except the last two
  token tiles, whose stores move to the then-idle sync ring.
- the final token tile runs ob-outer with quarter drains chained behind
  each bank's last matmul; the one before it uses half drains, so the
  tail is a short pipelined chain instead of serial DVE work.
"""

import numpy as np
import ml_dtypes

import concourse.bass as bass
import concourse.mybir as mybir
import concourse.tile as tile
from concourse.bass_utils import run_bass_kernel_spmd

N_CORES = 8
N_TOKENS = 65536
D_IN = 1024
D_OUT = 2048
N_EXPERTS = 16
E_PER_CORE = N_EXPERTS // N_CORES        # 2
TOK_PER_CORE = N_TOKENS // N_CORES       # 8192
TOK_PER_EXPERT = N_TOKENS // N_EXPERTS   # 4096

P = 128
KB = D_IN // P            # 8 contraction blocks of 128
KP = KB // 2              # 4 DoubleRow k-pairs (256 contraction each)
TS = 512                  # token stripe
OB = 512                  # out-feature block (one PSUM bank)
N_OB = D_OUT // OB        # 4
N_TSUB = TS // P          # 4
STRIPES_PER_EXPERT = TOK_PER_EXPERT // TS  # 8
N_STRIPES = E_PER_CORE * STRIPES_PER_EXPERT  # 16

S_X = 4.0                 # keeps x (std 1) in e4m3 normal range
S_W = 128.0               # keeps W (std ~0.0054) out of e4m3 subnormals
OUT_SCALE = 1.0 / (S_X * S_W)

N_WARMUP_MM = 6

F32 = mybir.dt.float32
F16 = mybir.dt.float16
F8 = mybir.dt.float8e4
NP_F8 = ml_dtypes.float8_e4m3

DR = mybir.MatmulPerfMode.DoubleRow
SILU = mybir.ActivationFunctionType.Silu
ADD = mybir.AluOpType.add


def _split_multi_waits(nc):
    """This container's walrus encodes at most ONE sync-wait per instruction;
    hoist extras onto single-wait NoOps inserted just before, same engine."""
    for fn in nc.m.functions:
        for bb in fn.blocks:
            insts = list(bb.instructions)
            out = []
            dirty = False
            for inst in insts:
                si = inst.sync_info
                waits = list(si.on_wait) if si and si.on_wait else []
                if len(waits) > 1:
                    dirty = True
                    for j, w in enumerate(waits[:-1]):
                        nop = mybir.InstNoOp(
                            name=f"{inst.name}-prewait{j}", ins=[], outs=[]
                        )
                        nop.engine = inst.engine
                        nop.sync_info = mybir.SyncInfo(on_wait=[w], on_update=[])
                        out.append(nop)
                    inst.sync_info = mybir.SyncInfo(
                        on_wait=[waits[-1]],
                        on_update=list(si.on_update) if si.on_update else [],
                    )
                out.append(inst)
            if dirty:
                bb.instructions = out


def build_kernel():
    nc = bass.Bass()
    # xt pre-striped on host: [stripe, partition, kb, t] -> 4KB lines
    xt = nc.dram_tensor("xt", [N_STRIPES, P, KB, TS], F8, kind="ExternalInput")
    # w pre-laid per k-pair in SBUF tile order -> 4KB lines
    w = nc.dram_tensor("w", [E_PER_CORE, KP, P, 2 * D_OUT], F8,
                       kind="ExternalInput")
    bb = nc.dram_tensor("bb", [E_PER_CORE, P, D_OUT], F16, kind="ExternalInput")
    y = nc.dram_tensor("y", [TOK_PER_CORE, D_OUT], F16, kind="ExternalOutput")

    with tile.TileContext(nc) as tc:
        with (
            tc.tile_pool(name="persist", bufs=1) as persist,
            tc.tile_pool(name="xp", bufs=3) as xp,
            tc.tile_pool(name="outs", bufs=3) as outs,
            tc.tile_pool(name="psum", bufs=2, space="PSUM") as psump,
        ):
            # --- PE warm-up: matmuls over zeroed scratch, no DMA deps ---
            zs = persist.tile([P, 2, TS], F8, name="warm_src")
            nc.vector.memset(zs[:], 0.0)
            ps_warm = psump.tile([P, N_OB, OB], F32, tag="ps", name="ps_warm")
            for i in range(N_WARMUP_MM):
                nc.tensor.matmul(
                    ps_warm[:, i % N_OB, :],
                    lhsT=zs[:, :, 0:P],
                    rhs=zs[:],
                    start=True, stop=True,
                    perf_mode=DR,
                    skip_group_check=True,
                )

            # --- critical preload: expert-0 weights + x stripe 0 ---
            x8_tiles = {}
            x8_tiles[0] = xp.tile([P, KB, TS], F8, tag="x8", name="x8_s0")

            b_sb = [
                persist.tile([P, N_OB, OB], F16, name=f"bias_{e}")
                for e in range(E_PER_CORE)
            ]
            w8 = [
                [
                    persist.tile([P, 2, D_OUT], F8, name=f"w8_{e}_{h}")
                    for h in range(KP)
                ]
                for e in range(E_PER_CORE)
            ]

            def load_w(e, h, eng):
                eng.dma_start(w8[e][h][:], w[e, h].rearrange("p (j o) -> p j o",
                                                            j=2))

            load_w(0, 0, nc.scalar)
            nc.sync.dma_start(x8_tiles[0][:], xt[0])
            load_w(0, 1, nc.scalar)
            load_w(0, 2, nc.sync)
            load_w(0, 3, nc.sync)
            # bias-e0 deferred: the tiny gpsimd copy waits (in-order
            # sequencer) until w-e0 kp1 has landed, keeping the early head's
            # queue slots for the critical weight stream; bias still arrives
            # well before the first drain needs it.
            bias_gate = persist.tile([P, 64], F8, name="bias_gate")
            nc.gpsimd.tensor_copy(bias_gate[:], w8[0][1][:, 0, 0:64])
            nc.gpsimd.dma_start(b_sb[0][:], bb[0])

            def load_expert(e):
                for h in range(KP):
                    eng = nc.scalar if h % 2 == 0 else nc.sync
                    load_w(e, h, eng)
                nc.gpsimd.dma_start(b_sb[e][:], bb[e])

            for e in range(E_PER_CORE):
                for s in range(STRIPES_PER_EXPERT):
                    g = e * STRIPES_PER_EXPERT + s
                    t0 = g * TS
                    if g in x8_tiles:
                        x8t = x8_tiles[g]
                    else:
                        x8t = xp.tile([P, KB, TS], F8, tag="x8", name="x8")
                        nc.sync.dma_start(x8t[:], xt[g])

                    def lhsT_of(kp, tsub):
                        return x8t[:, 2 * kp:2 * kp + 2, tsub * P:(tsub + 1) * P]

                    for tsub in range(N_TSUB):
                        last = g == N_STRIPES - 1 and tsub == N_TSUB - 1
                        rows = slice(t0 + tsub * P, t0 + (tsub + 1) * P)
                        ps = psump.tile([P, N_OB, OB], F32, tag="ps", name="ps")
                        for kp in range(KP):
                            for ob in range(N_OB):
                                nc.tensor.matmul(
                                    ps[:, ob, :],
                                    lhsT=lhsT_of(kp, tsub),
                                    rhs=w8[e][kp][:, :, ob * OB:(ob + 1) * OB],
                                    start=(kp == 0),
                                    stop=(kp == KP - 1),
                                    perf_mode=DR,
                                )
                        if not last:
                            y_sb = outs.tile([P, N_OB, OB], F32, tag="ysb",
                                             name="ysb")
                            nc.vector.tensor_tensor(y_sb[:], ps[:], b_sb[e][:],
                                                    ADD)
                            y_act = outs.tile([P, N_OB, OB], F16, tag="yact",
                                              name="yact")
                            nc.scalar.activation(y_act[:], y_sb[:], SILU,
                                                 scale=OUT_SCALE)
                            nc.scalar.dma_start(y[rows, :], y_act[:])
                        else:
                            # final tile: per-ob pipelined drain so the tail
                            # is DVE->ACT->small store chains, not one big op
                            y_act = outs.tile([P, N_OB, OB], F16, tag="yact",
                                              name="yact_f")
                            for ob in range(N_OB):
                                y_sb = outs.tile([P, OB], F32, tag="ysbq",
                                                 name="ysbq")
                                nc.vector.tensor_tensor(
                                    y_sb[:], ps[:, ob, :], b_sb[e][:, ob, :],
                                    ADD)
                                nc.scalar.activation(
                                    y_act[:, ob, :], y_sb[:], SILU,
                                    scale=OUT_SCALE)
                                # all final stores on the now-idle sync ring:
                                # keeps the ACT sequencer's silu chain free
                                # of 600ns store-issue slices
                                nc.sync.dma_start(
                                    y[rows, ob * OB:(ob + 1) * OB],
                                    y_act[:, ob, :])
                    if g == 0:
                        load_expert(1)

    _split_multi_waits(nc)
    return nc


_NC_CACHE = None


def _get_nc():
    global _NC_CACHE
    if _NC_CACHE is None:
        _NC_CACHE = build_kernel()
    return _NC_CACHE


def _in_maps(sorted_features, routing_matrix, routing_bias):
    maps = []
    for c in range(N_CORES):
        rows = slice(c * TOK_PER_CORE, (c + 1) * TOK_PER_CORE)
        es = slice(c * E_PER_CORE, (c + 1) * E_PER_CORE)
        # [stripe, partition, kb, t]: element (s,p,kb,t) = S_X*X_c[s*TS+t, kb*P+p]
        xt_c = np.ascontiguousarray(
            (sorted_features[rows] * S_X)
            .reshape(N_STRIPES, TS, KB, P)
            .transpose(0, 3, 2, 1)
            .astype(NP_F8)
        )
        # [e, kp, p, j*D_OUT+o] = S_W * W_e[(2*kp+j)*128+p, o]
        w_c = np.ascontiguousarray(
            (routing_matrix[:, :, es] * S_W)
            .transpose(2, 0, 1)                      # [E, D_IN, D_OUT]
            .reshape(E_PER_CORE, KP, 2, P, D_OUT)
            .transpose(0, 1, 3, 2, 4)                # [E, KP, P, 2, D_OUT]
            .reshape(E_PER_CORE, KP, P, 2 * D_OUT)
            .astype(NP_F8)
        )
        b_c = np.ascontiguousarray(
            np.broadcast_to(
                (routing_bias[:, es] * (S_X * S_W)).T[:, None, :],
                (E_PER_CORE, P, D_OUT),
            ).astype(np.float16)
        )
        maps.append({"xt": xt_c, "w": w_c, "bb": b_c})
    return maps


def run(sorted_features, routing_matrix, routing_bias, **run_kwargs):
    nc = _get_nc()
    maps = _in_maps(sorted_features, routing_matrix, routing_bias)
    res = run_bass_kernel_spmd(nc, maps, core_ids=list(range(N_CORES)), **run_kwargs)
    out = np.concatenate(
        [np.asarray(res.results[c]["y"]) for c in range(N_CORES)], axis=0
    ).astype(np.float32)
    return out, res


def kernel(sorted_features, expert_ids_sorted, routing_matrix, routing_bias):
    assert sorted_features.shape == (N_TOKENS, D_IN)
    assert routing_matrix.shape == (D_IN, D_OUT, N_EXPERTS)
    assert routing_bias.shape == (D_OUT, N_EXPERTS)
    out, _ = run(
        np.asarray(sorted_features, dtype=np.float32),
        np.asarray(routing_matrix, dtype=np.float32),
        np.asarray(routing_bias, dtype=np.float32),
    )
    return out


# revision 11
# speedup vs baseline: 1.0083x; 1.0083x over previous
"""MoE expert-collection grouped GEMM for Trainium2, expert-parallel over 8
NeuronCores, fp8 DoubleRow matmuls.

Problem (hardcoded shapes):
  sorted_features  [65536, 1024] f32   tokens sorted by expert, 4096/expert
  expert_ids_sorted[65536] i32         unused: split is static equal-count
  routing_matrix   [1024, 2048, 16] f32
  routing_bias     [2048, 16] f32
  out = silu(x_e @ W_e + b_e) per expert  -> [65536, 2048] f32

Sharding: expert-parallel, 2 experts (= 8192 contiguous sorted tokens) per
core. Host-side dispatch hands each core its token block transposed
(feature-major) and scaled by S_X in fp8 e4m3, its 2 experts' weights scaled
by S_W in fp8 e4m3 (DRAM pre-laid in SBUF tile order: 4KB per-partition
lines), and bias pre-scaled by S_X*S_W broadcast to 128 partitions in fp16.
The matmul term has std ~0.17 vs bias std 1.0, so fp8 noise lands well
under the 2e-2 gate.

Device pipeline per core: DoubleRow fp8 matmuls (two 128-row k-blocks per
instruction, 2 MACs/cell/cycle) accumulating fp32 in PSUM, k-pair-outer /
out-block-inner so one stationary x tile serves 4 matmuls. A tsub's 4 PSUM
banks accumulate while the other 4 drain through one batched DVE add
(+prescaled fp16 bias) and one batched ACT Silu (scale=1/(S_X*S_W))
writing fp16; host upcasts to fp32.

Schedule shaping:
- 6 zero-matmul warmups flip the PE HAM clock-gate to 2.4GHz while the
  critical preload streams, so real matmuls start warm.
- weight/x loads are need-ordered across the two HWDGE rings; expert-0
  bias is deferred behind a dependency on w-kp1 so its descriptors don't
  steal head-window queue slots from the critical weight stream.
- y stores ride the scalar ring (a store on the sync ring blocks the
  in-order sequencer and delays x prefetch issue) except the last two
  token tiles, whose stores move to the then-idle sync ring.
- the final token tile runs ob-outer with quarter drains chained behind
  each bank's last matmul; the one before it uses half drains, so the
  tail is a short pipelined chain instead of serial DVE work.
"""

import numpy as np
import ml_dtypes

import concourse.bass as bass
import concourse.mybir as mybir
import concourse.tile as tile
from concourse.bass_utils import run_bass_kernel_spmd

N_CORES = 8
N_TOKENS = 65536
D_IN = 1024
D_OUT = 2048
N_EXPERTS = 16
E_PER_CORE = N_EXPERTS // N_CORES        # 2
TOK_PER_CORE = N_TOKENS // N_CORES       # 8192
TOK_PER_EXPERT = N_TOKENS // N_EXPERTS   # 4096

P = 128
KB = D_IN // P            # 8 contraction blocks of 128
KP = KB // 2              # 4 DoubleRow k-pairs (256 contraction each)
TS = 512                  # token stripe
OB = 512                  # out-feature block (one PSUM bank)
N_OB = D_OUT // OB        # 4
N_TSUB = TS // P          # 4
STRIPES_PER_EXPERT = TOK_PER_EXPERT // TS  # 8
N_STRIPES = E_PER_CORE * STRIPES_PER_EXPERT  # 16

S_X = 4.0                 # keeps x (std 1) in e4m3 normal range
S_W = 128.0               # keeps W (std ~0.0054) out of e4m3 subnormals
OUT_SCALE = 1.0 / (S_X * S_W)

N_WARMUP_MM = 8

F32 = mybir.dt.float32
F16 = mybir.dt.float16
F8 = mybir.dt.float8e4
NP_F8 = ml_dtypes.float8_e4m3

DR = mybir.MatmulPerfMode.DoubleRow
SILU = mybir.ActivationFunctionType.Silu
ADD = mybir.AluOpType.add


def _split_multi_waits(nc):
    """This container's walrus encodes at most ONE sync-wait per instruction;
    hoist extras onto single-wait NoOps inserted just before, same engine."""
    for fn in nc.m.functions:
        for bb in fn.blocks:
            insts = list(bb.instructions)
            out = []
            dirty = False
            for inst in insts:
                si = inst.sync_info
                waits = list(si.on_wait) if si and si.on_wait else []
                if len(waits) > 1:
                    dirty = True
                    for j, w in enumerate(waits[:-1]):
                        nop = mybir.InstNoOp(
                            name=f"{inst.name}-prewait{j}", ins=[], outs=[]
                        )
                        nop.engine = inst.engine
                        nop.sync_info = mybir.SyncInfo(on_wait=[w], on_update=[])
                        out.append(nop)
                    inst.sync_info = mybir.SyncInfo(
                        on_wait=[waits[-1]],
                        on_update=list(si.on_update) if si.on_update else [],
                    )
                out.append(inst)
            if dirty:
                bb.instructions = out


def build_kernel():
    nc = bass.Bass()
    # xt pre-striped on host: [stripe, partition, kb, t] -> 4KB lines
    xt = nc.dram_tensor("xt", [N_STRIPES, P, KB, TS], F8, kind="ExternalInput")
    # w pre-laid per k-pair in SBUF tile order -> 4KB lines
    w = nc.dram_tensor("w", [E_PER_CORE, KP, P, 2 * D_OUT], F8,
                       kind="ExternalInput")
    bb = nc.dram_tensor("bb", [E_PER_CORE, P, D_OUT], F16, kind="ExternalInput")
    y = nc.dram_tensor("y", [TOK_PER_CORE, D_OUT], F16, kind="ExternalOutput")

    with tile.TileContext(nc) as tc:
        with (
            tc.tile_pool(name="persist", bufs=1) as persist,
            tc.tile_pool(name="xp", bufs=3) as xp,
            tc.tile_pool(name="outs", bufs=3) as outs,
            tc.tile_pool(name="psum", bufs=2, space="PSUM") as psump,
        ):
            # --- PE warm-up: matmuls over zeroed scratch, no DMA deps ---
            zs = persist.tile([P, 2, TS], F8, name="warm_src")
            nc.vector.memset(zs[:], 0.0)
            ps_warm = psump.tile([P, N_OB, OB], F32, tag="ps", name="ps_warm")
            for i in range(N_WARMUP_MM):
                nc.tensor.matmul(
                    ps_warm[:, i % N_OB, :],
                    lhsT=zs[:, :, 0:P],
                    rhs=zs[:],
                    start=True, stop=True,
                    perf_mode=DR,
                    skip_group_check=True,
                )

            # --- critical preload: expert-0 weights + x stripe 0 ---
            x8_tiles = {}
            x8_tiles[0] = xp.tile([P, KB, TS], F8, tag="x8", name="x8_s0")

            b_sb = [
                persist.tile([P, N_OB, OB], F16, name=f"bias_{e}")
                for e in range(E_PER_CORE)
            ]
            w8 = [
                [
                    persist.tile([P, 2, D_OUT], F8, name=f"w8_{e}_{h}")
                    for h in range(KP)
                ]
                for e in range(E_PER_CORE)
            ]

            def load_w(e, h, eng):
                eng.dma_start(w8[e][h][:], w[e, h].rearrange("p (j o) -> p j o",
                                                            j=2))

            load_w(0, 0, nc.scalar)
            nc.sync.dma_start(x8_tiles[0][:], xt[0])
            load_w(0, 1, nc.scalar)
            load_w(0, 2, nc.sync)
            load_w(0, 3, nc.sync)
            # bias-e0 deferred: the tiny gpsimd copy waits (in-order
            # sequencer) until w-e0 kp1 has landed, keeping the early head's
            # queue slots for the critical weight stream; bias still arrives
            # well before the first drain needs it.
            bias_gate = persist.tile([P, 64], F8, name="bias_gate")
            nc.gpsimd.tensor_copy(bias_gate[:], w8[0][1][:, 0, 0:64])
            nc.gpsimd.dma_start(b_sb[0][:], bb[0])

            def load_expert(e):
                for h in range(KP):
                    eng = nc.scalar if h % 2 == 0 else nc.sync
                    load_w(e, h, eng)
                nc.gpsimd.dma_start(b_sb[e][:], bb[e])

            for e in range(E_PER_CORE):
                for s in range(STRIPES_PER_EXPERT):
                    g = e * STRIPES_PER_EXPERT + s
                    t0 = g * TS
                    if g in x8_tiles:
                        x8t = x8_tiles[g]
                    else:
                        x8t = xp.tile([P, KB, TS], F8, tag="x8", name="x8")
                        nc.sync.dma_start(x8t[:], xt[g])

                    def lhsT_of(kp, tsub):
                        return x8t[:, 2 * kp:2 * kp + 2, tsub * P:(tsub + 1) * P]

                    for tsub in range(N_TSUB):
                        last = g == N_STRIPES - 1 and tsub == N_TSUB - 1
                        rows = slice(t0 + tsub * P, t0 + (tsub + 1) * P)
                        ps = psump.tile([P, N_OB, OB], F32, tag="ps", name="ps")
                        # final tile runs ob-outer so each bank's accumulation
                        # group closes early and its quarter drain can start
                        # while later banks still accumulate
                        loops = (
                            [(kp, ob) for kp in range(KP) for ob in range(N_OB)]
                            if not last else
                            [(kp, ob) for ob in range(N_OB) for kp in range(KP)]
                        )
                        for kp, ob in loops:
                            nc.tensor.matmul(
                                ps[:, ob, :],
                                lhsT=lhsT_of(kp, tsub),
                                rhs=w8[e][kp][:, :, ob * OB:(ob + 1) * OB],
                                start=(kp == 0),
                                stop=(kp == KP - 1),
                                perf_mode=DR,
                            )
                        if not last:
                            y_sb = outs.tile([P, N_OB, OB], F32, tag="ysb",
                                             name="ysb")
                            nc.vector.tensor_tensor(y_sb[:], ps[:], b_sb[e][:],
                                                    ADD)
                            y_act = outs.tile([P, N_OB, OB], F16, tag="yact",
                                              name="yact")
                            nc.scalar.activation(y_act[:], y_sb[:], SILU,
                                                 scale=OUT_SCALE)
                            nc.scalar.dma_start(y[rows, :], y_act[:])
                        else:
                            # final tile: per-ob pipelined drain so the tail
                            # is DVE->ACT->small store chains, not one big op
                            y_act = outs.tile([P, N_OB, OB], F16, tag="yact",
                                              name="yact_f")
                            for ob in range(N_OB):
                                y_sb = outs.tile([P, OB], F32, tag="ysbq",
                                                 name="ysbq")
                                nc.vector.tensor_tensor(
                                    y_sb[:], ps[:, ob, :], b_sb[e][:, ob, :],
                                    ADD)
                                nc.scalar.activation(
                                    y_act[:, ob, :], y_sb[:], SILU,
                                    scale=OUT_SCALE)
                                # all final stores on the now-idle sync ring:
                                # keeps the ACT sequencer's silu chain free
                                # of 600ns store-issue slices
                                nc.sync.dma_start(
                                    y[rows, ob * OB:(ob + 1) * OB],
                                    y_act[:, ob, :])
                    if g == 0:
                        load_expert(1)

    _split_multi_waits(nc)
    return nc


_NC_CACHE = None


def _get_nc():
    global _NC_CACHE
    if _NC_CACHE is None:
        _NC_CACHE = build_kernel()
    return _NC_CACHE


def _in_maps(sorted_features, routing_matrix, routing_bias):
    maps = []
    for c in range(N_CORES):
        rows = slice(c * TOK_PER_CORE, (c + 1) * TOK_PER_CORE)
        es = slice(c * E_PER_CORE, (c + 1) * E_PER_CORE)
        # [stripe, partition, kb, t]: element (s,p,kb,t) = S_X*X_c[s*TS+t, kb*P+p]
        xt_c = np.ascontiguousarray(
            (sorted_features[rows] * S_X)
            .reshape(N_STRIPES, TS, KB, P)
            .transpose(0, 3, 2, 1)
            .astype(NP_F8)
        )
        # [e, kp, p, j*D_OUT+o] = S_W * W_e[(2*kp+j)*128+p, o]
        w_c = np.ascontiguousarray(
            (routing_matrix[:, :, es] * S_W)
            .transpose(2, 0, 1)                      # [E, D_IN, D_OUT]
            .reshape(E_PER_CORE, KP, 2, P, D_OUT)
            .transpose(0, 1, 3, 2, 4)                # [E, KP, P, 2, D_OUT]
            .reshape(E_PER_CORE, KP, P, 2 * D_OUT)
            .astype(NP_F8)
        )
        b_c = np.ascontiguousarray(
            np.broadcast_to(
                (routing_bias[:, es] * (S_X * S_W)).T[:, None, :],
                (E_PER_CORE, P, D_OUT),
            ).astype(np.float16)
        )
        maps.append({"xt": xt_c, "w": w_c, "bb": b_c})
    return maps


def run(sorted_features, routing_matrix, routing_bias, **run_kwargs):
    nc = _get_nc()
    maps = _in_maps(sorted_features, routing_matrix, routing_bias)
    res = run_bass_kernel_spmd(nc, maps, core_ids=list(range(N_CORES)), **run_kwargs)
    out = np.concatenate(
        [np.asarray(res.results[c]["y"]) for c in range(N_CORES)], axis=0
    ).astype(np.float32)
    return out, res


def kernel(sorted_features, expert_ids_sorted, routing_matrix, routing_bias):
    assert sorted_features.shape == (N_TOKENS, D_IN)
    assert routing_matrix.shape == (D_IN, D_OUT, N_EXPERTS)
    assert routing_bias.shape == (D_OUT, N_EXPERTS)
    out, _ = run(
        np.asarray(sorted_features, dtype=np.float32),
        np.asarray(routing_matrix, dtype=np.float32),
        np.asarray(routing_bias, dtype=np.float32),
    )
    return out
